# revision 26
# baseline (speedup 1.0000x reference)
# Trainium2 Bass kernel for DirectionalStockGNN (2-layer GATv2 + residual head).
#
# Layout/strategy (bf16 edge-major pipeline):
#  - Nodes are sharded dst-contiguously: core c owns nodes [c*6272, (c+1)*6272).
#    Edges (with mean-attr self loops) are sorted by dst; each core processes
#    windows of 124 consecutive dst nodes.
#  - The per-layer "gather table" holds one 256-elem bf16 row per node:
#    [xl(128) | 1.0 | s1 | pad]; s1 = 0.2*att@xl is the linear score part of
#    the source node (the per-dst linear part cancels in the segment softmax,
#    and the per-edge part s3 = 0.2*att@(ea@We) is host-precomputed).
#    Table rows live in DRAM in a partition-major layout (node n -> row
#    (c*128 + j%128)*49 + j//128, j = n - 6272*c) so table builds write
#    2KB-contiguous per partition, and layer 2's table is assembled by an
#    AllGather of per-core slices.
#  - Host ships pure graph-structure blobs: gather indices (int16 wrap-16),
#    the dst one-hot stationary [124 one-hot ; 4 ea^T] per edge (blobOH) and
#    the edge-major one-hot mask (blobMK), so no one-hots are built on-chip.
#  - Per window: dma_gather fetches xg [128e, kw, 256] (edge-major),
#    m = onehotT @ [xr;We] (PE) + xg (PE identity matmul) accumulated in PSUM,
#    z = relu(m) (ACT), score = sum_f 0.8*att_f*z (DVE mult+reduce),
#    ee = exp(score + s1[src] + s3[e]) (ACT), see = mask*ee (DVE),
#    node-major agg[124,130] += see^T @ xg (PE); the table's ones column
#    yields the softmax denominator in column 128 for free.
#  - Epilogue: divide by den, +bias, ELU (exact, via exp/min); layer 1
#    transposes h into feature-major hT for layer 2's tables; layer 2 adds
#    the residual x and reduces against Wfc (the ELU's -1 is folded into bfc).
#  - Windows are software-pipelined 2 deep with deep tile pools (gathers run
#    ~5 windows ahead); fat blob loads go on the scalar-engine DMA ring so
#    gather index loads on the sync ring are never head-of-line blocked.
#  - Gather desc-gen on the Pool engine is the bottleneck (~8.4ns/idx on one
#    SWDGE queue). num_swdge_queues=4 with the queue rotated PER WINDOW gives
#    ~2.3x desc-gen throughput. Rotation granularity matters for correctness:
#    consumers wait only on the LAST gather call's DMA semaphore (the
#    framework elides dominated waits assuming same-queue FIFO order), so all
#    calls of a window MUST share one queue; per-call rotation corrupts
#    results and per-section rotation NaNs. CH>8 (>65 FIFO entries in flight
#    per queue) hangs the device; keep CH=8.
#  - phase2 of window w-2 is drained BEFORE phase1 of window w so epilogue
#    ops sit ahead in the engine queues and xg slots free before the next
#    gather needs them; scores are clamped at 60 before exp so stale pad-row
#    data can never produce inf*0 NaNs.

import math
import os

import ml_dtypes
import numpy as np

BF = ml_dtypes.bfloat16

D = 128
DE = 4
WIN = 124
NEG = 0.2
NCORES = 8

N_NODES = 50000
NPC = 6272            # padded nodes per core (49*128)
NSLOT = NPC // 128    # 49
NTAB = NCORES * NPC   # 50176 table rows
HALF = NTAB // 2      # 25088, int16 gather range split
NW = math.ceil(NPC / WIN)  # 51 windows/core
ELEM = 256            # bf16 elements per table row (512B)
CH = 8                # gather blocks per dma_gather call


def _row_of_node(n):
    """table row for node n (vectorized)."""
    c = n // NPC
    j = n - c * NPC
    return (c * 128 + (j % 128)) * NSLOT + j // 128


def _wrap16(idx):
    n = idx.shape[0]
    assert n % 16 == 0
    iw = np.zeros((16, n // 16), np.int16)
    iw[np.arange(n) % 16, np.arange(n) // 16] = idx
    return np.tile(iw, (8, 1))  # [128, n//16]


def build_host_data(edge_index, edge_attr, ins):
    N = N_NODES
    src0 = np.asarray(edge_index[0], dtype=np.int64)
    dst0 = np.asarray(edge_index[1], dtype=np.int64)
    ea = np.asarray(edge_attr, dtype=np.float32)

    # self loops with mean edge_attr per dst (PyG fill_value='mean')
    sums = np.zeros((N, DE), np.float32)
    np.add.at(sums, dst0, ea)
    cnts = np.bincount(dst0, minlength=N).astype(np.float32)
    loop_attr = sums / np.maximum(cnts, 1.0)[:, None]

    src = np.concatenate([src0, np.arange(N, dtype=np.int64)])
    dst = np.concatenate([dst0, np.arange(N, dtype=np.int64)])
    eaa = np.concatenate([ea, loop_attr], axis=0)

    order = np.argsort(dst, kind="stable")
    src_s = src[order]
    dst_s = dst[order]
    ea_s = eaa[order]

    rows_s = _row_of_node(src_s)

    # per-edge linear score parts s3 = 0.2*att@(ea@We), per layer
    q1 = NEG * (np.asarray(ins["W1e"], np.float32) @ np.asarray(ins["att1"], np.float32))
    q2 = NEG * (np.asarray(ins["W2e"], np.float32) @ np.asarray(ins["att2"], np.float32))
    s3_1 = ea_s @ q1
    s3_2 = ea_s @ q2

    # window edge ranges; common (max-over-cores) block grid so the SPMD
    # program is identical on every core
    starts = np.minimum(np.arange(NW + 1) * WIN, NPC)
    bounds = np.empty((NCORES, NW + 1), np.int64)
    for c in range(NCORES):
        lo = min(c * NPC, N)
        tops = np.minimum(lo + starts, N)
        bounds[c] = np.searchsorted(dst_s, tops)

    nlo = np.empty((NCORES, NW), np.int64)
    nhi = np.empty((NCORES, NW), np.int64)
    for c in range(NCORES):
        for w in range(NW):
            a, b = bounds[c, w], bounds[c, w + 1]
            m = rows_s[a:b] < HALF
            nlo[c, w] = int(m.sum())
            nhi[c, w] = int(b - a - nlo[c, w])
    KWLO = np.maximum(np.ceil(nlo.max(axis=0) / 128.0).astype(np.int64), 1)
    KWHI = np.ceil(nhi.max(axis=0) / 128.0).astype(np.int64)
    KW = KWLO + KWHI
    koff = np.zeros(NW + 1, np.int64)
    for w in range(NW):
        koff[w + 1] = koff[w] + int(KW[w])
    KTOT = int(koff[NW])

    blobI = np.zeros((NCORES, 128, 8 * KTOT), np.int16)
    blobS31 = np.zeros((NCORES, 128, KTOT), BF)
    blobS32 = np.zeros((NCORES, 128, KTOT), BF)
    # dst one-hot stationary [124 one-hot ; 4 ea^T] per edge column
    blobOH = np.zeros((NCORES, 128, KTOT * 128), BF)
    # edge-major dst one-hot mask (for see build)
    blobMK = np.zeros((NCORES, 128, KTOT * WIN), BF)
    nidxlo = np.zeros(NW, np.int64)
    nidxhi = np.zeros(NW, np.int64)

    for c in range(NCORES):
        for w in range(NW):
            a, b = bounds[c, w], bounds[c, w + 1]
            kwlo, kwhi = int(KWLO[w]), int(KWHI[w])
            kw = kwlo + kwhi
            ew = kw * 128
            base = min(c * NPC, N) + int(starts[w])
            sw_rows = rows_s[a:b]
            dw = (dst_s[a:b] - base).astype(np.float32)
            eaw = ea_s[a:b]
            s31w = s3_1[a:b]
            s32w = s3_2[a:b]
            mlo = sw_rows < HALF

            rowp = np.zeros(ew, np.int64)
            drel = np.full(ew, 127.0, np.float32)
            eap = np.zeros((ew, DE), np.float32)
            s31p = np.zeros(ew, np.float32)
            s32p = np.zeros(ew, np.float32)
            nl = int(mlo.sum())
            rowp[:nl] = sw_rows[mlo]
            drel[:nl] = dw[mlo]
            eap[:nl] = eaw[mlo]
            s31p[:nl] = s31w[mlo]
            s32p[:nl] = s32w[mlo]
            b0 = kwlo * 128
            nh = int((~mlo).sum())
            rowp[b0:b0 + nh] = sw_rows[~mlo]
            drel[b0:b0 + nh] = dw[~mlo]
            eap[b0:b0 + nh] = eaw[~mlo]
            s31p[b0:b0 + nh] = s31w[~mlo]
            s32p[b0:b0 + nh] = s32w[~mlo]
            rowp[b0 + nh:] = HALF  # hi-section pads -> rel row 0 (real data)

            ko = int(koff[w])
            ilo = _wrap16(rowp[:b0].astype(np.int16))
            blobI[c, :, 8 * ko: 8 * ko + 8 * kwlo] = ilo
            if kwhi:
                ihi = _wrap16((rowp[b0:] - HALF).astype(np.int16))
                blobI[c, :, 8 * (ko + kwlo): 8 * (ko + kw)] = ihi
            blobS31[c, :, ko:ko + kw] = s31p.reshape(kw, 128).T.astype(BF)
            blobS32[c, :, ko:ko + kw] = s32p.reshape(kw, 128).T.astype(BF)
            di = drel.astype(np.int64)
            real = drel < 124.5
            ecols = np.arange(ew)
            oh = np.zeros((128, ew), BF)
            oh[di[real], ecols[real]] = 1.0
            oh[124:128, :] = eap.T.astype(BF)
            blobOH[c, :, 128 * ko: 128 * ko + ew] = oh
            mk = np.zeros((ew, WIN), BF)
            mk[ecols[real], di[real]] = 1.0
            blobMK[c, :, WIN * ko: WIN * ko + kw * WIN] = (
                mk.reshape(kw, 128, WIN).transpose(1, 0, 2).reshape(128, kw * WIN))
            nidxlo[w] = max(nidxlo[w], ((nlo[c, w] + 15) // 16) * 16)
            nidxhi[w] = max(nidxhi[w], ((nhi[c, w] + 15) // 16) * 16)

    sched = dict(
        KWLO=[int(v) for v in KWLO], KWHI=[int(v) for v in KWHI],
        koff=[int(v) for v in koff], KTOT=KTOT,
        NIDXLO=[int(v) for v in nidxlo], NIDXHI=[int(v) for v in nidxhi],
    )
    blobs = dict(blobI=blobI, blobS31=blobS31,
                 blobS32=blobS32, blobOH=blobOH, blobMK=blobMK)
    return sched, blobs


def build_consts(ins):
    f32 = np.float32
    c = {}
    x = np.asarray(ins["x"], f32)
    xTb = np.zeros((D, NTAB), BF)
    xTb[:, :N_NODES] = x.T.astype(BF)
    c["xTb"] = xTb
    for li in (1, 2):
        Wl = np.asarray(ins[f"W{li}l"], f32)
        Wr = np.asarray(ins[f"W{li}r"], f32)
        We = np.asarray(ins[f"W{li}e"], f32)
        att = np.asarray(ins[f"att{li}"], f32)
        wlx = np.zeros((D, 130), f32)
        wlx[:, :128] = Wl
        wlx[:, 129] = NEG * (Wl @ att)  # s1 projection
        c[f"wlx{li}"] = wlx.astype(BF)
        c[f"wr{li}"] = Wr.astype(BF)
        c[f"wet{li}"] = np.tile(We.astype(BF)[:, None, :], (1, NW, 1)).reshape(4, NW * D)
        c[f"att08_{li}"] = np.tile(((1.0 - NEG) * att).astype(BF)[None, None, :], (D, 1, 1))

    c["wfcbc"] = np.tile(np.asarray(ins["Wfc"], f32).reshape(1, D).astype(BF), (WIN, 1))
    for li in (1, 2):
        c[f"bbr{li}"] = np.tile(np.asarray(ins[f"b{li}"], f32).reshape(1, D), (WIN, 1))
    c["identb"] = np.eye(D, dtype=f32).astype(BF)
    c["identf"] = np.eye(D, dtype=f32)
    return c


def build_program(sched, bfc_val):
    import concourse.bacc as bacc
    import concourse.mybir as mybir
    import concourse.tile as tile

    f32 = mybir.dt.float32
    bf16 = mybir.dt.bfloat16
    i16 = mybir.dt.int16
    Alu = mybir.AluOpType
    Act = mybir.ActivationFunctionType

    KWLO, KWHI = sched["KWLO"], sched["KWHI"]
    NIDXLO, NIDXHI = sched["NIDXLO"], sched["NIDXHI"]
    koff = sched["koff"]
    KTOT = sched["KTOT"]
    KW = [KWLO[w] + KWHI[w] for w in range(NW)]
    KWMAX = max(KW)
    EWMAX = KWMAX * 128
    HT = NW * WIN  # 6324

    maxphase = int(os.environ.get("GNN_MAXPHASE", "9"))
    nopipe = int(os.environ.get("GNN_NOPIPE", "0"))
    poison = int(os.environ.get("GNN_POISON", "0"))

    nc = bacc.Bacc(
        "TRN2", target_bir_lowering=False, debug=False,
        enable_asserts=False, num_devices=NCORES,
        num_swdge_queues=4,
    )

    # ---- I/O ----
    t_xTb = nc.dram_tensor("xTb", [D, NTAB], bf16, kind="ExternalInput")
    t_xT_own = nc.dram_tensor("xT_own", [D, NPC], bf16, kind="ExternalInput")
    t_x_own = nc.dram_tensor("x_own", [NPC, D], bf16, kind="ExternalInput")
    t_blobI = nc.dram_tensor("blobI", [128, 8 * KTOT], i16, kind="ExternalInput")
    t_blobS31 = nc.dram_tensor("blobS31", [128, KTOT], bf16, kind="ExternalInput")
    t_blobS32 = nc.dram_tensor("blobS32", [128, KTOT], bf16, kind="ExternalInput")
    t_blobOH = nc.dram_tensor("blobOH", [128, 128 * KTOT], bf16, kind="ExternalInput")
    t_blobMK = nc.dram_tensor("blobMK", [128, WIN * KTOT], bf16, kind="ExternalInput")

    cshapes = dict(
        wlx1=([D, 130], bf16), wlx2=([D, 130], bf16),
        wr1=([D, D], bf16), wr2=([D, D], bf16),
        wet1=([4, NW * D], bf16), wet2=([4, NW * D], bf16),
        att08_1=([D, 1, D], bf16), att08_2=([D, 1, D], bf16),
        bbr1=([WIN, D], f32), bbr2=([WIN, D], f32),
        wfcbc=([WIN, D], bf16), identb=([D, D], bf16), identf=([D, D], f32),
    )
    t_c = {k: nc.dram_tensor(k, sh, dt, kind="ExternalInput")
           for k, (sh, dt) in cshapes.items()}
    t_y = nc.dram_tensor("y", [NW, WIN], f32, kind="ExternalOutput")

    t_tab1 = nc.dram_tensor("tab1", [NTAB, ELEM], bf16, kind="Internal")
    t_t2own = nc.dram_tensor("t2own", [NPC, ELEM], bf16, kind="Internal")
    t_tab2 = nc.dram_tensor(
        "tab2", [NCORES, NPC, ELEM], bf16, kind="Internal",
        addr_space=("Shared" if NCORES > 1 else "Local"),
    )
    t_tab2l = nc.dram_tensor("tab2l", [NTAB, ELEM], bf16, kind="Internal")

    with tile.TileContext(nc) as tc:
        with (
            tc.tile_pool(name="cpool", bufs=1) as cpool,
            tc.tile_pool(name="sp", bufs=2) as sp,
            tc.tile_pool(name="spw", bufs=2) as spw,
            tc.tile_pool(name="spb", bufs=4) as spb,
            tc.tile_pool(name="spsee", bufs=4) as spsee,
            tc.tile_pool(name="xgp", bufs=5) as xgp,
            tc.tile_pool(name="pm", bufs=2, space="PSUM") as pm_pool,
            tc.tile_pool(name="pagg", bufs=2, space="PSUM") as pagg_pool,
            tc.tile_pool(name="ptr", bufs=2, space="PSUM") as ptr_pool,
            tc.tile_pool(name="paux", bufs=2, space="PSUM") as paux_pool,
        ):
            C = {}
            for k, (sh, dt) in cshapes.items():
                C[k] = cpool.tile(sh, dt, tag=f"c_{k}", name=f"c_{k}")
                nc.sync.dma_start(out=C[k][:], in_=t_c[k][:])

            mv = cpool.tile([D, NW, D], bf16, tag="mv", name="mv")
            nc.sync.dma_start(
                out=mv[124:128, :, :],
                in_=t_c["wet1"][:, :].rearrange("q (w d) -> q w d", w=NW, d=D))
            hT = cpool.tile([D, HT], bf16, tag="hT", name="hT")
            y_sb = cpool.tile([WIN, NW], f32, tag="y_sb", name="y_sb")
            nc.vector.memset(y_sb[:, :], 0.0)

            t1v = t_tab1[:, :].rearrange("(c p b) e -> c p (b e)",
                                         c=NCORES, p=128, b=NSLOT)
            t2ov = t_t2own[:, :].rearrange("(p b) e -> p (b e)", p=128, b=NSLOT)

            def table1():
                wlx = C["wlx1"]
                GW = 12  # table slots per write group
                for cc in range(NCORES):
                    for q0 in range(0, NSLOT, GW):
                        qn = min(GW, NSLOT - q0)
                        xb = sp.tile([D, GW * 128], bf16, tag="xb", name="xb")
                        col = cc * NPC + q0 * 128
                        nc.sync.dma_start(out=xb[:, :qn * 128],
                                          in_=t_xTb[:, col:col + qn * 128])
                        stg = sp.tile([D, GW, ELEM], bf16, tag="stg", name="stg")
                        for i0 in range(0, qn, 3):
                            inn = min(3, qn - i0)
                            pt = paux_pool.tile([D, 390], f32, tag="paux",
                                                name="pt")
                            for i in range(inn):
                                nc.tensor.matmul(
                                    out=pt[:, i * 130:(i + 1) * 130],
                                    lhsT=xb[:, (i0 + i) * 128:(i0 + i + 1) * 128],
                                    rhs=wlx[:, :], start=True, stop=True)
                            nc.vector.tensor_copy(
                                out=stg[:, i0:i0 + inn, 0:130],
                                in_=pt[:, 0:inn * 130].rearrange(
                                    "p (g e) -> p g e", g=inn, e=130))
                        nc.vector.memset(stg[:, 0:qn, 128:129], 1.0)
                        nc.sync.dma_start(
                            out=t1v[cc, :, q0 * ELEM:(q0 + qn) * ELEM],
                            in_=stg[:, 0:qn, :])

            def table2():
                wlx = C["wlx2"]
                for g0 in range(0, NSLOT, 2):
                    gn = min(2, NSLOT - g0)
                    pt = paux_pool.tile([D, 260], f32, tag="paux", name="pt2")
                    stg = sp.tile([D, 2, ELEM], bf16, tag="stg", name="stg2")
                    for i in range(gn):
                        b = g0 + i
                        nc.tensor.matmul(
                            out=pt[:, i * 130:(i + 1) * 130],
                            lhsT=hT[:, b * 128:(b + 1) * 128], rhs=wlx[:, :],
                            start=True, stop=True)
                    nc.vector.tensor_copy(
                        out=stg[:, 0:gn, 0:130],
                        in_=pt[:, 0:gn * 130].rearrange("p (g e) -> p g e", g=gn, e=130))
                    nc.vector.memset(stg[:, 0:gn, 128:129], 1.0)
                    for i in range(gn):
                        b = g0 + i
                        nc.sync.dma_start(
                            out=t2ov[:, b * ELEM:(b + 1) * ELEM],
                            in_=stg[:, i, :])

            def xr_phase2():
                wr = C["wr2"]
                nc.sync.dma_start(
                    out=mv[124:128, :, :],
                    in_=t_c["wet2"][:, :].rearrange("q (w d) -> q w d", w=NW, d=D))
                for w in range(NW):
                    pxt = paux_pool.tile([D, 260], f32, tag="paux", name="pxr2")
                    pxr = pxt[0:WIN, 0:D]
                    nc.tensor.matmul(out=pxr,
                                     lhsT=hT[:, w * WIN:w * WIN + WIN],
                                     rhs=wr[:, :], start=True, stop=True)
                    nc.vector.tensor_copy(out=mv[0:WIN, w, :], in_=pxr)

            def xr_phase1():
                wr = C["wr1"]
                for w in range(NW):
                    wn = min(WIN, NPC - w * WIN)
                    pxt = paux_pool.tile([D, 260], f32, tag="paux", name="pxr1")
                    pxr = pxt[0:WIN, 0:D]
                    nc.tensor.matmul(out=pxr[0:wn, :],
                                     lhsT=xob[:, w * WIN:w * WIN + wn],
                                     rhs=wr[:, :], start=True, stop=True)
                    nc.vector.tensor_copy(out=mv[0:wn, w, :], in_=pxr[0:wn, :])

            xob = cpool.tile([D, NPC], bf16, tag="xob", name="xob")
            nc.sync.dma_start(out=xob[:, :], in_=t_xT_own[:, :])
            for _ in range(5):
                xg0 = xgp.tile([128, KWMAX, ELEM], bf16, tag="xg", name="xg0")
                nc.vector.memset(xg0[:, :, :], 0.0)

            # ---------------- edge pass ----------------
            def win_blobs(layer, w):
                kwlo, kwhi = KWLO[w], KWHI[w]
                kw = kwlo + kwhi
                ew = kw * 128
                ko = koff[w]
                t_s3 = t_blobS31 if layer == 1 else t_blobS32
                it = spb.tile([128, 8 * KWMAX], i16, tag="it", name="it")
                nc.sync.dma_start(out=it[:, :8 * kw],
                                  in_=t_blobI[:, 8 * ko:8 * (ko + kw)])
                s3 = spb.tile([128, KWMAX], bf16, tag="s3", name="s3")
                nc.sync.dma_start(out=s3[:, :kw], in_=t_s3[:, ko:ko + kw])
                rhs1h = spb.tile([128, EWMAX], bf16, tag="rhs1h", name="rhs1h")
                nc.scalar.dma_start(out=rhs1h[:, :ew],
                                    in_=t_blobOH[:, 128 * ko:128 * ko + ew])
                see = spsee.tile([128, KWMAX, WIN], bf16, tag="see", name="see")
                nc.scalar.dma_start(out=see[:, :kw, :],
                                    in_=t_blobMK[:, WIN * ko:WIN * (ko + kw)]
                                    .rearrange("p (k j) -> p k j", k=kw, j=WIN))
                return dict(it=it, s3=s3, rhs1h=rhs1h, see=see)

            def win_phase1(layer, w, bl):
                """gathers + m + scores + see for window w; returns tiles."""
                kwlo, kwhi = KWLO[w], KWHI[w]
                kw = kwlo + kwhi
                ew = kw * 128
                ko = koff[w]
                tv = (t_tab1[:, :] if layer == 1 else
                      t_tab2[:, :, :].rearrange("c n e -> (c n) e"))
                att08 = C[f"att08_{layer}"]
                it, s3, rhs1h, see = bl["it"], bl["s3"], bl["rhs1h"], bl["see"]

                xg = xgp.tile([128, KWMAX, ELEM], bf16, tag="xg", name="xg")
                in_lo = tv[0:HALF, :]
                in_hi = tv[HALF:NTAB, :]
                nlo_w, nhi_w = NIDXLO[w], NIDXHI[w]
                for g0 in range(0, kwlo, CH):
                    n_call = min(CH * 128, nlo_w - g0 * 128)
                    gn = (n_call + 127) // 128
                    nc.gpsimd.dma_gather(
                        out_ap=xg[:, g0:g0 + gn, :], in_ap=in_lo,
                        idxs_ap=it[:, 8 * g0:8 * g0 + ((n_call + 15) // 16)],
                        num_idxs=n_call, num_idxs_reg=n_call,
                        elem_size=ELEM, queue_num=qrr[0] % 4)
                for g0 in range(0, kwhi, CH):
                    n_call = min(CH * 128, nhi_w - g0 * 128)
                    gn = (n_call + 127) // 128
                    nc.gpsimd.dma_gather(
                        out_ap=xg[:, kwlo + g0:kwlo + g0 + gn, :], in_ap=in_hi,
                        idxs_ap=it[:, 8 * (kwlo + g0):
                                   8 * (kwlo + g0) + ((n_call + 15) // 16)],
                        num_idxs=n_call, num_idxs_reg=n_call,
                        elem_size=ELEM, queue_num=qrr[0] % 4)
                qrr[0] += 1

                z = spw.tile([128, KWMAX * 128], bf16, tag="z", name="z")
                for t0 in range(0, kw, 4):
                    nb = min(4, kw - t0)
                    T = nb * 128
                    pm = pm_pool.tile([128, 512], f32, tag="pm", name="pm")
                    for cb in range(nb):
                        blk = t0 + cb
                        nc.tensor.matmul(
                            out=pm[:, cb * 128:(cb + 1) * 128],
                            lhsT=rhs1h[:, blk * 128:(blk + 1) * 128],
                            rhs=mv[:, w, :], start=True, stop=True)
                    m2 = spw.tile([128, 512], bf16, tag="m2", name="m2")
                    nc.vector.tensor_tensor(
                        out=m2[:, :T].rearrange("p (k f) -> p k f", k=nb, f=128),
                        in0=pm[:, :T].rearrange("p (k f) -> p k f", k=nb, f=128),
                        in1=xg[:, t0:t0 + nb, 0:128], op=Alu.add)
                    nc.scalar.activation(
                        out=z[:, t0 * 128:t0 * 128 + T], in_=m2[:, :T],
                        func=Act.Relu)
                if maxphase < 3:
                    return None
                zs = spw.tile([128, KWMAX, 128], bf16, tag="zs", name="zs")
                nc.vector.tensor_tensor(
                    out=zs[:, :kw, :],
                    in0=z[:, :kw * 128].rearrange("p (k f) -> p k f", k=kw, f=128),
                    in1=att08[:, :, :].to_broadcast([128, kw, 128]),
                    op=Alu.mult)
                sc = spw.tile([128, KWMAX], f32, tag="sc", name="sc")
                nc.vector.tensor_reduce(
                    out=sc[:, :kw], in_=zs[:, :kw, :],
                    axis=mybir.AxisListType.X, op=Alu.add)
                nc.vector.tensor_tensor(
                    out=sc[:, :kw], in0=sc[:, :kw], in1=xg[:, :kw, 129],
                    op=Alu.add)
                nc.vector.tensor_tensor(
                    out=sc[:, :kw], in0=sc[:, :kw], in1=s3[:, :kw], op=Alu.add)
                # clamp: pad-edge scores computed from stale xg data must not
                # reach exp-overflow (inf * 0-mask = NaN)
                nc.vector.tensor_scalar(out=sc[:, :kw], in0=sc[:, :kw],
                                        scalar1=60.0, scalar2=None, op0=Alu.min)
                eew = spw.tile([128, KWMAX, 1], bf16, tag="eew", name="eew")
                nc.scalar.activation(out=eew[:, :kw, 0], in_=sc[:, :kw],
                                     func=Act.Exp)
                nc.vector.tensor_tensor(
                    out=see[:, :kw, :], in0=see[:, :kw, :],
                    in1=eew[:, :kw, :].to_broadcast([128, kw, WIN]),
                    op=Alu.mult)
                return dict(xg=xg, see=see, kw=kw)

            def win_phase2(layer, w, st):
                if st is None or maxphase < 4:
                    return
                kw = st["kw"]
                xg, see = st["xg"], st["see"]
                # node-major aggregation; col 128 (the table's ones column)
                # accumulates the softmax denominator for free
                pagg = pagg_pool.tile([WIN, 130], f32, tag="pagg", name="pagg")
                for blk in range(kw):
                    nc.tensor.matmul(
                        out=pagg[:, :], lhsT=see[:, blk, :],
                        rhs=xg[:, blk, 0:130],
                        start=(blk == 0), stop=(blk == kw - 1))
                if maxphase < 5:
                    return
                # epilogue: divide, bias, ELU (all node-major [124, 128])
                rec = sp.tile([WIN, 1], f32, tag="rec", name="rec")
                nc.vector.tensor_scalar(out=rec[:, :], in0=pagg[:, 128:129],
                                        scalar1=1e-30, scalar2=None, op0=Alu.max)
                nc.vector.reciprocal(out=rec[:, :], in_=rec[:, :])
                hw = sp.tile([WIN, D], f32, tag="hw", name="hw")
                nc.vector.tensor_scalar(out=hw[:, :], in0=pagg[:, 0:128],
                                        scalar1=rec[:, :], scalar2=None,
                                        op0=Alu.mult)
                nc.vector.tensor_tensor(out=hw[:, :], in0=hw[:, :],
                                        in1=C[f"bbr{layer}"][:, :], op=Alu.add)
                tmin = sp.tile([WIN, D], f32, tag="tmin", name="tmin")
                nc.vector.tensor_scalar(out=tmin[:, :], in0=hw[:, :],
                                        scalar1=0.0, scalar2=None, op0=Alu.min)
                uexp = sp.tile([WIN, D], f32, tag="uexp", name="uexp")
                nc.scalar.activation(out=uexp[:, :], in_=tmin[:, :], func=Act.Exp)
                nc.vector.tensor_tensor(out=hw[:, :], in0=hw[:, :],
                                        in1=tmin[:, :], op=Alu.subtract)
                if layer == 1:
                    nc.vector.tensor_tensor(out=hw[:, :], in0=hw[:, :],
                                            in1=uexp[:, :], op=Alu.add)
                    ptr = ptr_pool.tile([D, WIN], f32, tag="ptr", name="ptr")
                    nc.tensor.matmul(out=ptr[:, :], lhsT=hw[:, :],
                                     rhs=C["identf"][0:WIN, 0:WIN],
                                     is_transpose=True, start=True, stop=True)
                    nc.scalar.activation(out=hT[:, w * WIN:(w + 1) * WIN],
                                         in_=ptr[:, :], func=Act.Copy,
                                         bias=-1.0)
                    if maxphase >= 6:
                        # build table-2 slots whose hT inputs are complete, so
                        # the AllGather can start right at the end of layer 1
                        while (t2b[0] + 1) * 128 <= (w + 1) * WIN:
                            b = t2b[0]
                            pt2 = paux_pool.tile([D, 260], f32, tag="paux",
                                                 name="pt2i")
                            nc.tensor.matmul(
                                out=pt2[:, 0:130],
                                lhsT=hT[:, b * 128:(b + 1) * 128],
                                rhs=C["wlx2"][:, :], start=True, stop=True)
                            st2 = sp.tile([D, ELEM], bf16, tag="st2",
                                          name="st2")
                            nc.vector.tensor_copy(out=st2[:, 0:130],
                                                  in_=pt2[:, 0:130])
                            nc.vector.memset(st2[:, 128:129], 1.0)
                            nc.sync.dma_start(
                                out=t2ov[:, b * ELEM:(b + 1) * ELEM],
                                in_=st2[:, :])
                            t2b[0] += 1
                else:
                    wn = min(WIN, NPC - w * WIN)
                    nc.vector.tensor_tensor(out=hw[:, :], in0=hw[:, :],
                                            in1=uexp[:, :], op=Alu.add)
                    xrow = sp.tile([WIN, D], bf16, tag="xrow", name="xrow")
                    nc.sync.dma_start(out=xrow[0:wn, :],
                                      in_=t_x_own[w * WIN:w * WIN + wn, :])
                    res = sp.tile([WIN, D], bf16, tag="res", name="res")
                    # the ELU's -1 is folded into bfc on the host
                    nc.vector.tensor_tensor(out=res[:, :], in0=hw[:, :],
                                            in1=xrow[:, :], op=Alu.add)
                    ysc = sp.tile([WIN, D], f32, tag="ysc", name="ysc")
                    nc.vector.tensor_tensor(out=ysc[:, :], in0=res[:, :],
                                            in1=C["wfcbc"][:, :], op=Alu.mult)
                    nc.vector.tensor_reduce(
                        out=y_sb[:, w:w + 1], in_=ysc[:, :],
                        axis=mybir.AxisListType.X, op=Alu.add)

            def edge_pass(layer):
                if nopipe:
                    for w in range(NW):
                        st = win_phase1(layer, w, win_blobs(layer, w))
                        win_phase2(layer, w, st)
                else:
                    pend = []
                    bl = win_blobs(layer, 0)
                    for w in range(NW):
                        nbl = win_blobs(layer, w + 1) if w + 1 < NW else None
                        # drain phase2 BEFORE phase1 so the epilogue ops sit
                        # ahead of the new window's ops in each engine queue —
                        # xg slots free sooner and gathers don't stall on WAR
                        if len(pend) > 1:
                            pw, pst = pend.pop(0)
                            win_phase2(layer, pw, pst)
                        pend.append((w, win_phase1(layer, w, bl)))
                        bl = nbl
                    for pw, pst in pend:
                        win_phase2(layer, pw, pst)

            # ---------------- main ----------------
            t2b = [0]
            qrr = [0]
            if poison:
                # write huge values into internal DRAM read surfaces; any
                # read that races ahead of the proper producer turns into a
                # deterministic NaN/inf instead of a heisenbug
                pz = cpool.tile([128, 4 * ELEM], bf16, tag="pz", name="pz")
                nc.vector.memset(pz[:, :], 3.0e38)
                t1pv = t_tab1[:, :].rearrange("(c p b) e -> c p (b e)",
                                              c=NCORES, p=128, b=NSLOT)
                t2pv = t_tab2[:, :, :].rearrange("c (p b) e -> c p (b e)",
                                                 p=128, b=NSLOT)
                for cc in range(NCORES):
                    for q0 in range(0, NSLOT, 4):
                        qn = min(4, NSLOT - q0)
                        if poison & 1:
                            nc.scalar.dma_start(
                                out=t1pv[cc, :, q0 * ELEM:(q0 + qn) * ELEM],
                                in_=pz[:, :qn * ELEM])
                        if poison & 2:
                            nc.scalar.dma_start(
                                out=t2pv[cc, :, q0 * ELEM:(q0 + qn) * ELEM],
                                in_=pz[:, :qn * ELEM])
            table1()
            if maxphase >= 1:
                xr_phase1()
            if maxphase >= 2:
                edge_pass(1)
            if maxphase >= 6:
                if NCORES > 1:
                    nc.gpsimd.collective_compute(
                        "AllGather", mybir.AluOpType.bypass,
                        replica_groups=[list(range(NCORES))],
                        ins=[t_t2own[:, :]],
                        outs=[t_tab2[:, :, :]],
                    )
                else:
                    nc.sync.dma_start(out=t_tab2[0, :, :], in_=t_t2own[:, :])
            if maxphase >= 7:
                xr_phase2()
            if maxphase >= 8:
                edge_pass(2)
            nc.vector.tensor_scalar(out=y_sb[:, :], in0=y_sb[:, :],
                                    scalar1=float(bfc_val), scalar2=None,
                                    op0=Alu.add)
            pyt = ptr_pool.tile([D, WIN], f32, tag="ptr", name="pyt")
            nc.tensor.matmul(out=pyt[0:NW, :], lhsT=y_sb[:, :],
                             rhs=C["identf"][0:WIN, 0:WIN],
                             is_transpose=True, start=True, stop=True)
            yst = sp.tile([NW, WIN], f32, tag="yst", name="yst")
            nc.scalar.copy(out=yst[:, :], in_=pyt[0:NW, :])
            nc.sync.dma_start(out=t_y[:, :], in_=yst[:, :])

    nc.compile()
    return nc


def prepare(inputs, ncores=NCORES):
    sched, blobs = build_host_data(inputs["edge_index"], inputs["edge_attr"], inputs)
    consts = build_consts(inputs)
    bfc_val = float(np.asarray(inputs["bfc"]).reshape(-1)[0])
    bfc_val -= float(np.asarray(inputs["Wfc"], np.float32).sum())
    nc = build_program(sched, bfc_val)
    xT = np.asarray(inputs["x"], np.float32).T
    in_maps = []
    for c in range(ncores):
        m = {k: v for k, v in consts.items()}
        xo = np.zeros((D, NPC), BF)
        lo = c * NPC
        hi = min(N_NODES, lo + NPC)
        if hi > lo:
            xo[:, :hi - lo] = xT[:, lo:hi].astype(BF)
        m["xT_own"] = xo
        xn = np.zeros((NPC, D), BF)
        if hi > lo:
            xn[:hi - lo, :] = xT[:, lo:hi].T.astype(BF)
        m["x_own"] = xn
        for k in ("blobI", "blobS31", "blobS32", "blobOH", "blobMK"):
            m[k] = np.ascontiguousarray(blobs[k][c])
        in_maps.append(m)
    return nc, in_maps, sched


def kernel(**inputs) -> np.ndarray:
    nc, in_maps, sched = prepare(inputs)
    from concourse.bass_utils import run_bass_kernel_spmd

    res = run_bass_kernel_spmd(nc, in_maps, core_ids=list(range(NCORES)))
    ys = []
    for c in range(NCORES):
        lo = c * NPC
        n = min(N_NODES, lo + NPC) - lo
        if n > 0:
            ys.append(res.results[c]["y"].reshape(-1)[:n])
    y = np.concatenate(ys)[:N_NODES].astype(np.float32).reshape(N_NODES, 1)
    return y

